# revision 11
# baseline (speedup 1.0000x reference)
"""BioLinearAttention (ELU+1 linear attention) on 8 TRN2 NeuronCores.

Sharding: token-parallel. The (B, T) = (4, 4096) grid flattens to 16384 rows;
each core owns 2048 contiguous rows (core c holds batch c//2's half). Each core
computes k/v projections for its rows, accumulates partial kv = k'^T v and
k_sum directly in PSUM across all 16 token tiles, then a pairwise AllReduce
(cores 2b, 2b+1 share batch b) completes the per-batch kv / k_sum. The q
projection for all rows runs while the collective is in flight. Stage C
computes den = q'.k_sum via block-diagonal 2-head matmuls, pre-scales
q~ = q' / den on the DVE, then y^T = kv^T_blockdiag @ q~ (K=128 2-head packed)
and the output projection.

All matmul operands are bf16 (inputs cast host-side); PSUM accumulation stays
fp32. The output is written bf16 and cast back to fp32 on host. DVE element
wise ops run on bf16 SBUF tiles to hit the 2x/4x DVE fast paths; reciprocal
stays fp32 (reciprocal_approx_fast requires it).
"""

import sys
import types

import numpy as np

B, T, C = 4, 4096, 1024
H, D = 16, 64
N_CORES = 8
ROWS = B * T
RPC = ROWS // N_CORES  # rows per core: 2048
NT = RPC // 128  # 128-token tiles per core: 16
NST = RPC // 512  # 512-token super-tiles per core: 4

_CACHE = {}


def _ensure_hook_shim():
    """bass_utils imports antenv.axon_hooks when BASS_TRACE is set; the image
    lacks that module. Provide a no-op shim unless one is already installed."""
    if "antenv.axon_hooks" in sys.modules:
        return
    try:
        import antenv
    except ImportError:
        return
    mod = types.ModuleType("antenv.axon_hooks")
    mod._hook = None
    mod.set_axon_ntff_profile_hook = lambda h: setattr(mod, "_hook", h)
    mod.get_axon_ntff_profile_hook = lambda: mod._hook
    sys.modules["antenv.axon_hooks"] = mod
    antenv.axon_hooks = mod


def _build(with_bias):
    key = ("nc", with_bias)
    if key in _CACHE:
        return _CACHE[key]

    import concourse.bacc as bacc
    import concourse.mybir as mybir
    from concourse.tile import TileContext

    F32 = mybir.dt.float32
    BF16 = mybir.dt.bfloat16
    AF = mybir.ActivationFunctionType

    nc = bacc.Bacc("TRN2", num_devices=N_CORES, debug=False)

    xt = nc.dram_tensor("xt", [C, RPC], BF16, kind="ExternalInput")
    wkvt = nc.dram_tensor("wkvt", [C, 2 * C], BF16, kind="ExternalInput")
    wqt = nc.dram_tensor("wqt", [C, C], BF16, kind="ExternalInput")
    wct = nc.dram_tensor("wct", [C, C], BF16, kind="ExternalInput")
    if with_bias:
        bkv = nc.dram_tensor("bkv", [1, 2 * C], BF16, kind="ExternalInput")
        bq = nc.dram_tensor("bq", [1, C], BF16, kind="ExternalInput")
        bc = nc.dram_tensor("bc", [1, C], BF16, kind="ExternalInput")
    out = nc.dram_tensor("out", [RPC, C], BF16, kind="ExternalOutput")
    # kv (rows 0:64) | k_sum (row 64) partials, layout [d, (h, e)]
    cc_in = nc.dram_tensor("cc_in", [D + 1, C], F32, kind="Internal")
    cc_out = nc.dram_tensor("cc_out", [D + 1, C], F32, kind="Internal")
    groups = [[0, 1], [2, 3], [4, 5], [6, 7]]

    with TileContext(nc) as tc:
        with (
            tc.tile_pool(name="const", bufs=1) as cst,
            tc.tile_pool(name="wts", bufs=1) as wtp,
            tc.tile_pool(name="xres", bufs=1) as xrp,
            tc.tile_pool(name="qres", bufs=1) as qrp,
            tc.tile_pool(name="kvres", bufs=1) as kvp,
        ):
            # ---- persistent SBUF ----
            x_sb = xrp.tile([128, 8, RPC], BF16)  # x^T, c-chunk major
            wkv_sb = wtp.tile([128, 8, 2 * C], BF16)
            wq_sb = wtp.tile([128, 8, C], BF16)
            wc_sb = wtp.tile([128, 8, C], BF16)
            qn_sb = qrp.tile([128, 8, RPC], BF16)  # q' (ELU+1), later q~ in place
            kv2 = kvp.tile([128, 8, 128], BF16)  # block-diag per head pair
            krep2 = kvp.tile([128, 8, 128], BF16)  # block-diag ksum-replicated
            kvt_sb = kvp.tile([D + 1, 8, 128], F32)  # collective result
            ksum_bf = kvp.tile([1, 8, 128], BF16)
            onecol = cst.tile([128, 1], BF16)
            ones_row = cst.tile([1, 512], BF16)
            if with_bias:
                bkv_sb = cst.tile([1, 2 * C], BF16)
                bq_sb = cst.tile([1, C], BF16)
                bc_sb = cst.tile([1, C], BF16)
                nc.sync.dma_start(bkv_sb[:], bkv.ap())
                nc.sync.dma_start(bq_sb[:], bq.ap())
                nc.sync.dma_start(bc_sb[:], bc.ap())

            x_re = xt.ap().rearrange("(c p) t -> p c t", p=128)
            wkv_re = wkvt.ap().rearrange("(c p) n -> p c n", p=128)

            # per-ic granularity for tile 0's deps so the first k-proj matmul
            # can start as soon as its own slice lands (ic-ordered)
            for ic in range(8):
                nc.sync.dma_start(x_sb[:, ic, 0:128], x_re[:, ic, 0:128])
                nc.sync.dma_start(
                    wkv_sb[:, ic, 0:512], wkv_re[:, ic, 0:512]
                )
            for ic in range(8):
                nc.sync.dma_start(
                    wkv_sb[:, ic, 512:1024], wkv_re[:, ic, 512:1024]
                )
            nc.sync.dma_start(x_sb[:, :, 128:256], x_re[:, :, 128:256])
            for nh in range(2, 4):
                nc.sync.dma_start(
                    wkv_sb[:, :, nh * 512 : (nh + 1) * 512],
                    wkv_re[:, :, nh * 512 : (nh + 1) * 512],
                )
            nc.sync.dma_start(x_sb[:, :, 256:512], x_re[:, :, 256:512])
            nc.vector.memset(onecol[:], 1.0)
            nc.vector.memset(ones_row[:], 1.0)
            nc.vector.memset(kv2[:], 0.0)
            nc.vector.memset(krep2[:], 0.0)
            for half in range(2):  # rest of x
                nc.sync.dma_start(
                    x_sb[:, :, 512 + half * 768 : 512 + (half + 1) * 768],
                    x_re[:, :, 512 + half * 768 : 512 + (half + 1) * 768],
                )
            nc.sync.dma_start(
                wq_sb[:], wqt.ap().rearrange("(c p) n -> p c n", p=128)
            )
            nc.sync.dma_start(
                wc_sb[:], wct.ap().rearrange("(c p) n -> p c n", p=128)
            )

            # ---------------- stage A: k/v proj, kv & k_sum PSUM accumulation
            with (
                tc.tile_pool(name="kv1", bufs=2) as kvp1,
                tc.tile_pool(name="el1", bufs=2) as el1,
                tc.tile_pool(name="ps1", bufs=6, space="PSUM") as ps1,
                tc.tile_pool(name="kvacc", bufs=1, space="PSUM") as kvap,
            ):
                kvacc = kvap.tile([D + 1, C], F32)

                prev = None
                for tt in range(NT + 1):
                    if prev is not None:
                        kq0, vq0, t0 = prev
                        st_acc = t0 == 0
                        sp_acc = t0 == NT - 1
                        for h in range(H):
                            # start=True marks the whole 2KB PSUM bank row
                            # pending-zero, so only the first head touching
                            # each bank may set it; later heads' first-tile
                            # writes land on pending bytes and init cleanly.
                            nc.tensor.matmul(
                                kvacc[0:D, h * D : (h + 1) * D],
                                lhsT=kq0[:, h * D : (h + 1) * D],
                                rhs=vq0[:, h * D : (h + 1) * D],
                                start=st_acc and h % 8 == 0,
                                stop=sp_acc,
                                skip_group_check=True,
                            )
                        for nh in range(2):
                            nc.tensor.matmul(
                                kvacc[D : D + 1, nh * 512 : (nh + 1) * 512],
                                lhsT=onecol[:],
                                rhs=kq0[:, nh * 512 : (nh + 1) * 512],
                                start=st_acc,
                                stop=sp_acc,
                                skip_group_check=True,
                            )
                    if tt == NT:
                        break
                    xtile = x_sb[:, :, tt * 128 : (tt + 1) * 128]
                    kq = kvp1.tile([128, C], BF16)
                    vq = kvp1.tile([128, C], BF16)
                    for half in range(2):  # 0 = k, 1 = v
                        for nh in range(2):
                            sl = slice(half * C + nh * 512, half * C + (nh + 1) * 512)
                            pk = ps1.tile([128, 512], F32)
                            for ic in range(8):
                                nc.tensor.matmul(
                                    pk[:],
                                    lhsT=xtile[:, ic, :],
                                    rhs=wkv_sb[:, ic, sl],
                                    start=(ic == 0),
                                    stop=(ic == 7 and not with_bias),
                                )
                            if with_bias:
                                nc.tensor.matmul(
                                    pk[:],
                                    lhsT=ones_row[0:1, 0:128],
                                    rhs=bkv_sb[0:1, sl],
                                    start=False,
                                    stop=True,
                                )
                            osl = slice(nh * 512, (nh + 1) * 512)
                            if half == 0:
                                # elu(x)+1 = relu(x) + exp(min(x, 0))
                                krelu = el1.tile([128, 512], BF16, tag="krelu")
                                nc.scalar.activation(krelu[:], pk[:], AF.Relu)
                                kmin = el1.tile([128, 512], BF16, tag="kmin")
                                nc.vector.tensor_scalar_min(kmin[:], pk[:], 0.0)
                                nc.scalar.activation(kmin[:], kmin[:], AF.Exp)
                                nc.vector.tensor_add(kq[:, osl], krelu[:], kmin[:])
                            else:
                                nc.scalar.copy(vq[:, osl], pk[:])
                    prev = (kq, vq, tt)

                # kickoff AllReduce of [kv | k_sum]
                kvs = el1.tile([D + 1, C], F32, tag="kvs", bufs=1)
                nc.scalar.copy(kvs[:], kvacc[:])
                nc.gpsimd.dma_start(cc_in.ap(), kvs[:])
                nc.gpsimd.collective_compute(
                    "AllReduce",
                    mybir.AluOpType.add,
                    replica_groups=groups,
                    ins=[cc_in.ap().opt()],
                    outs=[cc_out.ap().opt()],
                )
                nc.gpsimd.dma_start(
                    kvt_sb[:], cc_out.ap().rearrange("p (c n) -> p c n", c=8)
                )

            # ---------------- stage B: q projection (overlaps the collective)
            with (
                tc.tile_pool(name="el2", bufs=3) as el2,
                tc.tile_pool(name="zpool", bufs=3) as zp,
                tc.tile_pool(name="ytz", bufs=2) as ytzp,
                tc.tile_pool(name="osb", bufs=3) as osbp,
                tc.tile_pool(name="psq", bufs=2, space="PSUM") as psq,
                tc.tile_pool(name="psden", bufs=2, space="PSUM") as psden,
                tc.tile_pool(name="psy", bufs=2, space="PSUM") as psy,
                tc.tile_pool(name="pso", bufs=2, space="PSUM") as pso,
            ):
                for st in range(NST):
                    tsl = slice(st * 512, (st + 1) * 512)
                    for oc in range(8):
                        qp = psq.tile([128, 512], F32)
                        for ic in range(8):
                            nc.tensor.matmul(
                                qp[:],
                                lhsT=wq_sb[:, ic, oc * 128 : (oc + 1) * 128],
                                rhs=x_sb[:, ic, tsl],
                                start=(ic == 0),
                                stop=(ic == 7 and not with_bias),
                            )
                        if with_bias:
                            nc.tensor.matmul(
                                qp[:],
                                lhsT=bq_sb[0:1, oc * 128 : (oc + 1) * 128],
                                rhs=ones_row[0:1, :],
                                start=False,
                                stop=True,
                            )
                        # elu(x)+1 = relu(x) + exp(min(x, 0)); alternate the
                        # min between DVE and Act (exp(min(x,0)) =
                        # exp(-relu(-x))) to balance engine load
                        qrelu = el2.tile([128, 512], BF16, tag="qrelu")
                        nc.scalar.activation(qrelu[:], qp[:], AF.Relu)
                        qmin = el2.tile([128, 512], BF16, tag="qmin")
                        if oc % 2 == 0:
                            nc.vector.tensor_scalar_min(qmin[:], qp[:], 0.0)
                            nc.scalar.activation(qmin[:], qmin[:], AF.Exp)
                        else:
                            nc.scalar.activation(
                                qmin[:], qp[:], AF.Relu, scale=-1.0
                            )
                            nc.scalar.activation(
                                qmin[:], qmin[:], AF.Exp, scale=-1.0
                            )
                        nc.vector.tensor_add(
                            qn_sb[:, oc, tsl], qrelu[:], qmin[:]
                        )

                # ---- kv2 / krep2 assembly (waits on the collective) ----
                nc.scalar.copy(ksum_bf[:], kvt_sb[D : D + 1, :, :])
                krp = psden.tile([128, 8, 64], F32, tag="dps")
                for h in range(H):
                    po = (h % 2) * 64
                    # one bank: only the first write per partition half may
                    # set start (bank-granular pending-zero, as above)
                    nc.tensor.matmul(
                        krp[po : po + 64, h // 2, :],
                        lhsT=ksum_bf[0:1, h // 2, po : po + 64],
                        rhs=ones_row[0:1, 0:64],
                        start=h < 2,
                        stop=True,
                        skip_group_check=True,
                    )
                # scatter kv and krep into zero-padded block-diagonal pair form
                for po in range(2):  # even heads -> rows 0:64, odd -> 64:128
                    sl64 = slice(po * 64, po * 64 + 64)
                    nc.vector.tensor_copy(
                        kv2[sl64, :, sl64],
                        kvt_sb[0:D, :, po * 64 : po * 64 + 64],
                    )
                    nc.vector.tensor_copy(
                        krep2[sl64, :, sl64],
                        krp[sl64, :, :],
                    )

                # den for all supertiles; recip + in-place q~ = q' * z
                for st in range(NST):
                    tsl = slice(st * 512, (st + 1) * 512)
                    for j in range(8):
                        dps = psden.tile([128, 512], F32)
                        nc.tensor.matmul(
                            dps[:],
                            lhsT=krep2[:, j, :],
                            rhs=qn_sb[:, j, tsl],
                            start=True,
                            stop=True,
                        )
                        z32 = zp.tile([128, 512], F32, tag="z32")
                        nc.vector.reciprocal_approx_fast(z32[:], dps[:])
                        z16 = zp.tile([128, 512], BF16, tag="z16")
                        nc.scalar.copy(z16[:], z32[:])
                        nc.vector.tensor_mul(
                            qn_sb[:, j, tsl], qn_sb[:, j, tsl], z16[:]
                        )

                # ---- stage C: y^T = blockdiag(kv)^T q~, then c_proj ----
                for st in range(NST):
                    tsl = slice(st * 512, (st + 1) * 512)
                    ytz = ytzp.tile([128, 8, 512], BF16)
                    for j in range(8):
                        yps = psy.tile([128, 512], F32)
                        nc.tensor.matmul(
                            yps[:],
                            lhsT=kv2[:, j, :],
                            rhs=qn_sb[:, j, tsl],
                            start=True,
                            stop=True,
                        )
                        nc.scalar.copy(ytz[:, j, :], yps[:])
                    for k in range(4):
                        gt = st * 4 + k
                        for ch in range(2):
                            op2 = pso.tile([128, 512], F32)
                            for oc2 in range(8):
                                nc.tensor.matmul(
                                    op2[:],
                                    lhsT=ytz[:, oc2, k * 128 : (k + 1) * 128],
                                    rhs=wc_sb[:, oc2, ch * 512 : (ch + 1) * 512],
                                    start=(oc2 == 0),
                                    stop=(oc2 == 7 and not with_bias),
                                )
                            if with_bias:
                                nc.tensor.matmul(
                                    op2[:],
                                    lhsT=ones_row[0:1, 0:128],
                                    rhs=bc_sb[0:1, ch * 512 : (ch + 1) * 512],
                                    start=False,
                                    stop=True,
                                )
                            osb = osbp.tile([128, 512], BF16)
                            # DVE is idle during the c_proj window; Act is not
                            nc.vector.tensor_copy(osb[:], op2[:])
                            nc.sync.dma_start(
                                out.ap()[
                                    gt * 128 : (gt + 1) * 128,
                                    ch * 512 : (ch + 1) * 512,
                                ],
                                osb[:],
                            )

    nc.compile()
    _CACHE[key] = nc
    return nc


LAST_RESULT = None


def kernel(x, Wq, bq, Wk, bk, Wv, bv, Wc, bc):
    global LAST_RESULT
    _ensure_hook_shim()
    import ml_dtypes
    from concourse.bass_utils import run_bass_kernel_spmd

    BF = ml_dtypes.bfloat16

    bq = np.asarray(bq, np.float32)
    bk = np.asarray(bk, np.float32)
    bv = np.asarray(bv, np.float32)
    bc = np.asarray(bc, np.float32)
    with_bias = bool(bq.any() or bk.any() or bv.any() or bc.any())
    nc = _build(with_bias)

    x = np.ascontiguousarray(np.asarray(x, dtype=np.float32))
    xt_full = np.ascontiguousarray(x.reshape(ROWS, C).T.astype(BF))  # [C, ROWS]
    wkvt = np.ascontiguousarray(
        np.concatenate(
            [np.asarray(Wk, np.float32).T, np.asarray(Wv, np.float32).T], axis=1
        ).astype(BF)
    )
    wqt = np.ascontiguousarray(np.asarray(Wq, np.float32).T.astype(BF))
    wct = np.ascontiguousarray(np.asarray(Wc, np.float32).T.astype(BF))

    in_maps = []
    for c in range(N_CORES):
        m = {
            "xt": np.ascontiguousarray(xt_full[:, c * RPC : (c + 1) * RPC]),
            "wkvt": wkvt,
            "wqt": wqt,
            "wct": wct,
        }
        if with_bias:
            m["bkv"] = np.concatenate([bk, bv]).reshape(1, 2 * C).astype(BF)
            m["bq"] = bq.reshape(1, C).astype(BF)
            m["bc"] = bc.reshape(1, C).astype(BF)
        in_maps.append(m)

    res = run_bass_kernel_spmd(nc, in_maps, core_ids=list(range(N_CORES)))
    LAST_RESULT = res
    out = np.concatenate(
        [
            np.asarray(res.results[c]["out"]).astype(np.float32)
            for c in range(N_CORES)
        ],
        axis=0,
    )
    return out.reshape(B, T, C)


# revision 14
# speedup vs baseline: 1.0227x; 1.0227x over previous
"""BioLinearAttention (ELU+1 linear attention) on 8 TRN2 NeuronCores.

Sharding: token-parallel. The (B, T) = (4, 4096) grid flattens to 16384 rows;
each core owns 2048 contiguous rows (core c holds batch c//2's half). Each core
computes k/v projections for its rows, accumulates partial kv = k'^T v and
k_sum directly in PSUM across all 16 token tiles, then a pairwise AllReduce
(cores 2b, 2b+1 share batch b) completes the per-batch kv / k_sum. The q
projection for all rows runs while the collective is in flight. Stage C
computes den = q'.k_sum via block-diagonal 2-head matmuls, pre-scales
q~ = q' / den on the DVE, then y^T = kv^T_blockdiag @ q~ (K=128 2-head packed)
and the output projection.

All matmul operands are bf16 (inputs cast host-side); PSUM accumulation stays
fp32. The output is written bf16 and cast back to fp32 on host. DVE element
wise ops run on bf16 SBUF tiles to hit the 2x/4x DVE fast paths; reciprocal
stays fp32 (reciprocal_approx_fast requires it).
"""

import sys
import types

import numpy as np

B, T, C = 4, 4096, 1024
H, D = 16, 64
N_CORES = 8
ROWS = B * T
RPC = ROWS // N_CORES  # rows per core: 2048
NT = RPC // 128  # 128-token tiles per core: 16
NST = RPC // 512  # 512-token super-tiles per core: 4

_CACHE = {}


def _ensure_hook_shim():
    """bass_utils imports antenv.axon_hooks when BASS_TRACE is set; the image
    lacks that module. Provide a no-op shim unless one is already installed."""
    if "antenv.axon_hooks" in sys.modules:
        return
    try:
        import antenv
    except ImportError:
        return
    mod = types.ModuleType("antenv.axon_hooks")
    mod._hook = None
    mod.set_axon_ntff_profile_hook = lambda h: setattr(mod, "_hook", h)
    mod.get_axon_ntff_profile_hook = lambda: mod._hook
    sys.modules["antenv.axon_hooks"] = mod
    antenv.axon_hooks = mod


def _build(with_bias):
    key = ("nc", with_bias)
    if key in _CACHE:
        return _CACHE[key]

    import concourse.bacc as bacc
    import concourse.mybir as mybir
    from concourse.tile import TileContext

    F32 = mybir.dt.float32
    BF16 = mybir.dt.bfloat16
    AF = mybir.ActivationFunctionType

    nc = bacc.Bacc("TRN2", num_devices=N_CORES, debug=False)

    xt = nc.dram_tensor("xt", [C, RPC], BF16, kind="ExternalInput")
    wkvt = nc.dram_tensor("wkvt", [C, 2 * C], BF16, kind="ExternalInput")
    wqt = nc.dram_tensor("wqt", [C, C], BF16, kind="ExternalInput")
    wct = nc.dram_tensor("wct", [C, C], BF16, kind="ExternalInput")
    if with_bias:
        bkv = nc.dram_tensor("bkv", [1, 2 * C], BF16, kind="ExternalInput")
        bq = nc.dram_tensor("bq", [1, C], BF16, kind="ExternalInput")
        bc = nc.dram_tensor("bc", [1, C], BF16, kind="ExternalInput")
    out = nc.dram_tensor("out", [RPC, C], BF16, kind="ExternalOutput")
    # kv (rows 0:64) | k_sum (row 64) partials, layout [d, (h, e)]
    cc_in = nc.dram_tensor("cc_in", [D + 1, C], F32, kind="Internal")
    cc_out = nc.dram_tensor("cc_out", [D + 1, C], F32, kind="Internal")
    groups = [[0, 1], [2, 3], [4, 5], [6, 7]]

    with TileContext(nc) as tc:
        with (
            tc.tile_pool(name="const", bufs=1) as cst,
            tc.tile_pool(name="wts", bufs=1) as wtp,
            tc.tile_pool(name="xres", bufs=1) as xrp,
            tc.tile_pool(name="qres", bufs=1) as qrp,
            tc.tile_pool(name="kvres", bufs=1) as kvp,
        ):
            # ---- persistent SBUF ----
            x_sb = xrp.tile([128, 8, RPC], BF16)  # x^T, c-chunk major
            wkv_sb = wtp.tile([128, 8, 2 * C], BF16)
            wq_sb = wtp.tile([128, 8, C], BF16)
            wc_sb = wtp.tile([128, 8, C], BF16)
            qn_sb = qrp.tile([128, 8, RPC], BF16)  # q' (ELU+1), later q~ in place
            kv2 = kvp.tile([128, 8, 128], BF16)  # block-diag per head pair
            krep2 = kvp.tile([128, 8, 128], BF16)  # block-diag ksum-replicated
            kvt_sb = kvp.tile([D + 1, 8, 128], F32)  # collective result
            ksum_bf = kvp.tile([1, 8, 128], BF16)
            onecol = cst.tile([128, 1], BF16)
            ones_row = cst.tile([1, 512], BF16)
            if with_bias:
                bkv_sb = cst.tile([1, 2 * C], BF16)
                bq_sb = cst.tile([1, C], BF16)
                bc_sb = cst.tile([1, C], BF16)
                nc.sync.dma_start(bkv_sb[:], bkv.ap())
                nc.sync.dma_start(bq_sb[:], bq.ap())
                nc.sync.dma_start(bc_sb[:], bc.ap())

            x_re = xt.ap().rearrange("(c p) t -> p c t", p=128)
            wkv_re = wkvt.ap().rearrange("(c p) n -> p c n", p=128)

            # per-ic granularity for tile 0's deps so the first k-proj matmul
            # can start as soon as its own slice lands (ic-ordered)
            for ic in range(8):
                nc.sync.dma_start(x_sb[:, ic, 0:128], x_re[:, ic, 0:128])
                nc.sync.dma_start(
                    wkv_sb[:, ic, 0:512], wkv_re[:, ic, 0:512]
                )
            for ic in range(8):
                nc.sync.dma_start(
                    wkv_sb[:, ic, 512:1024], wkv_re[:, ic, 512:1024]
                )
            nc.sync.dma_start(x_sb[:, :, 128:256], x_re[:, :, 128:256])
            for nh in range(2, 4):
                nc.sync.dma_start(
                    wkv_sb[:, :, nh * 512 : (nh + 1) * 512],
                    wkv_re[:, :, nh * 512 : (nh + 1) * 512],
                )
            nc.sync.dma_start(x_sb[:, :, 256:512], x_re[:, :, 256:512])
            nc.vector.memset(onecol[:], 1.0)
            nc.vector.memset(ones_row[:], 1.0)
            nc.vector.memset(kv2[:], 0.0)
            nc.vector.memset(krep2[:], 0.0)
            for half in range(2):  # rest of x
                nc.sync.dma_start(
                    x_sb[:, :, 512 + half * 768 : 512 + (half + 1) * 768],
                    x_re[:, :, 512 + half * 768 : 512 + (half + 1) * 768],
                )
            nc.sync.dma_start(
                wq_sb[:], wqt.ap().rearrange("(c p) n -> p c n", p=128)
            )
            nc.sync.dma_start(
                wc_sb[:], wct.ap().rearrange("(c p) n -> p c n", p=128)
            )

            # ---------------- stage A: k/v proj, kv & k_sum PSUM accumulation
            with (
                tc.tile_pool(name="kv1", bufs=2) as kvp1,
                tc.tile_pool(name="el1", bufs=2) as el1,
                tc.tile_pool(name="ps1", bufs=6, space="PSUM") as ps1,
                tc.tile_pool(name="kvacc", bufs=1, space="PSUM") as kvap,
            ):
                kvacc = kvap.tile([D + 1, C], F32)

                prev = None
                for tt in range(NT + 1):
                    if prev is not None:
                        kq0, vq0, t0 = prev
                        st_acc = t0 == 0
                        sp_acc = t0 == NT - 1
                        for h in range(H):
                            # start=True marks the whole 2KB PSUM bank row
                            # pending-zero, so only the first head touching
                            # each bank may set it; later heads' first-tile
                            # writes land on pending bytes and init cleanly.
                            nc.tensor.matmul(
                                kvacc[0:D, h * D : (h + 1) * D],
                                lhsT=kq0[:, h * D : (h + 1) * D],
                                rhs=vq0[:, h * D : (h + 1) * D],
                                start=st_acc and h % 8 == 0,
                                stop=sp_acc,
                                skip_group_check=True,
                            )
                        for nh in range(2):
                            nc.tensor.matmul(
                                kvacc[D : D + 1, nh * 512 : (nh + 1) * 512],
                                lhsT=onecol[:],
                                rhs=kq0[:, nh * 512 : (nh + 1) * 512],
                                start=st_acc,
                                stop=sp_acc,
                                skip_group_check=True,
                            )
                    if tt == NT:
                        break
                    xtile = x_sb[:, :, tt * 128 : (tt + 1) * 128]
                    kq = kvp1.tile([128, C], BF16)
                    vq = kvp1.tile([128, C], BF16)
                    for half in range(2):  # 0 = k, 1 = v
                        for nh in range(2):
                            sl = slice(half * C + nh * 512, half * C + (nh + 1) * 512)
                            pk = ps1.tile([128, 512], F32)
                            for ic in range(8):
                                nc.tensor.matmul(
                                    pk[:],
                                    lhsT=xtile[:, ic, :],
                                    rhs=wkv_sb[:, ic, sl],
                                    start=(ic == 0),
                                    stop=(ic == 7 and not with_bias),
                                )
                            if with_bias:
                                nc.tensor.matmul(
                                    pk[:],
                                    lhsT=ones_row[0:1, 0:128],
                                    rhs=bkv_sb[0:1, sl],
                                    start=False,
                                    stop=True,
                                )
                            osl = slice(nh * 512, (nh + 1) * 512)
                            if half == 0:
                                # elu(x)+1 = relu(x) + exp(min(x, 0))
                                krelu = el1.tile([128, 512], BF16, tag="krelu")
                                nc.scalar.activation(krelu[:], pk[:], AF.Relu)
                                kmin = el1.tile([128, 512], BF16, tag="kmin")
                                nc.vector.tensor_scalar_min(kmin[:], pk[:], 0.0)
                                nc.scalar.activation(kmin[:], kmin[:], AF.Exp)
                                nc.vector.tensor_add(kq[:, osl], krelu[:], kmin[:])
                            else:
                                nc.scalar.copy(vq[:, osl], pk[:])
                    prev = (kq, vq, tt)

                # kickoff AllReduce of [kv | k_sum]
                kvs = el1.tile([D + 1, C], F32, tag="kvs", bufs=1)
                nc.scalar.copy(kvs[:], kvacc[:])
                nc.gpsimd.dma_start(cc_in.ap(), kvs[:])
                nc.gpsimd.collective_compute(
                    "AllReduce",
                    mybir.AluOpType.add,
                    replica_groups=groups,
                    ins=[cc_in.ap().opt()],
                    outs=[cc_out.ap().opt()],
                )
                nc.gpsimd.dma_start(
                    kvt_sb[:], cc_out.ap().rearrange("p (c n) -> p c n", c=8)
                )

            # ---------------- stage B: q projection (overlaps the collective)
            with (
                tc.tile_pool(name="el2", bufs=3) as el2,
                tc.tile_pool(name="zpool", bufs=3) as zp,
                tc.tile_pool(name="ytz", bufs=2) as ytzp,
                tc.tile_pool(name="osb", bufs=3) as osbp,
                tc.tile_pool(name="psq", bufs=2, space="PSUM") as psq,
                tc.tile_pool(name="psden", bufs=2, space="PSUM") as psden,
                tc.tile_pool(name="psy", bufs=2, space="PSUM") as psy,
                tc.tile_pool(name="pso", bufs=2, space="PSUM") as pso,
            ):
                for st in range(NST):
                    tsl = slice(st * 512, (st + 1) * 512)
                    for oc in range(8):
                        qp = psq.tile([128, 512], F32)
                        for ic in range(8):
                            nc.tensor.matmul(
                                qp[:],
                                lhsT=wq_sb[:, ic, oc * 128 : (oc + 1) * 128],
                                rhs=x_sb[:, ic, tsl],
                                start=(ic == 0),
                                stop=(ic == 7 and not with_bias),
                            )
                        if with_bias:
                            nc.tensor.matmul(
                                qp[:],
                                lhsT=bq_sb[0:1, oc * 128 : (oc + 1) * 128],
                                rhs=ones_row[0:1, :],
                                start=False,
                                stop=True,
                            )
                        # elu(x)+1 = relu(x) + exp(min(x, 0))
                        qrelu = el2.tile([128, 512], BF16, tag="qrelu")
                        nc.scalar.activation(qrelu[:], qp[:], AF.Relu)
                        qmin = el2.tile([128, 512], BF16, tag="qmin")
                        nc.vector.tensor_scalar_min(qmin[:], qp[:], 0.0)
                        nc.scalar.activation(qmin[:], qmin[:], AF.Exp)
                        nc.vector.tensor_add(
                            qn_sb[:, oc, tsl], qrelu[:], qmin[:]
                        )

                # ---- kv2 / krep2 assembly (waits on the collective) ----
                nc.scalar.copy(ksum_bf[:], kvt_sb[D : D + 1, :, :])
                krp = psden.tile([128, 8, 64], F32, tag="dps")
                for h in range(H):
                    po = (h % 2) * 64
                    # one bank: only the first write per partition half may
                    # set start (bank-granular pending-zero, as above)
                    nc.tensor.matmul(
                        krp[po : po + 64, h // 2, :],
                        lhsT=ksum_bf[0:1, h // 2, po : po + 64],
                        rhs=ones_row[0:1, 0:64],
                        start=h < 2,
                        stop=True,
                        skip_group_check=True,
                    )
                # scatter kv and krep into zero-padded block-diagonal pair form
                for po in range(2):  # even heads -> rows 0:64, odd -> 64:128
                    sl64 = slice(po * 64, po * 64 + 64)
                    nc.vector.tensor_copy(
                        kv2[sl64, :, sl64],
                        kvt_sb[0:D, :, po * 64 : po * 64 + 64],
                    )
                    nc.vector.tensor_copy(
                        krep2[sl64, :, sl64],
                        krp[sl64, :, :],
                    )

                # ---- stage C: den -> q~ = q'/den -> y^T -> c_proj, software
                # pipelined so each supertile's den/recip/mul overlaps the
                # previous supertile's c_proj instead of bunching on the DVE
                def emit_den(st):
                    tsl = slice(st * 512, (st + 1) * 512)
                    for j in range(8):
                        dps = psden.tile([128, 512], F32)
                        nc.tensor.matmul(
                            dps[:],
                            lhsT=krep2[:, j, :],
                            rhs=qn_sb[:, j, tsl],
                            start=True,
                            stop=True,
                        )
                        z32 = zp.tile([128, 512], F32, tag="z32")
                        nc.vector.reciprocal_approx_fast(z32[:], dps[:])
                        z16 = zp.tile([128, 512], BF16, tag="z16")
                        nc.scalar.copy(z16[:], z32[:])
                        nc.vector.tensor_mul(
                            qn_sb[:, j, tsl], qn_sb[:, j, tsl], z16[:]
                        )

                emit_den(0)
                for st in range(NST):
                    tsl = slice(st * 512, (st + 1) * 512)
                    ytz = ytzp.tile([128, 8, 512], BF16)
                    for j in range(8):
                        yps = psy.tile([128, 512], F32)
                        nc.tensor.matmul(
                            yps[:],
                            lhsT=kv2[:, j, :],
                            rhs=qn_sb[:, j, tsl],
                            start=True,
                            stop=True,
                        )
                        nc.scalar.copy(ytz[:, j, :], yps[:])
                    if st + 1 < NST:
                        emit_den(st + 1)
                    for k in range(4):
                        gt = st * 4 + k
                        for ch in range(2):
                            op2 = pso.tile([128, 512], F32)
                            for oc2 in range(8):
                                nc.tensor.matmul(
                                    op2[:],
                                    lhsT=ytz[:, oc2, k * 128 : (k + 1) * 128],
                                    rhs=wc_sb[:, oc2, ch * 512 : (ch + 1) * 512],
                                    start=(oc2 == 0),
                                    stop=(oc2 == 7 and not with_bias),
                                )
                            if with_bias:
                                nc.tensor.matmul(
                                    op2[:],
                                    lhsT=ones_row[0:1, 0:128],
                                    rhs=bc_sb[0:1, ch * 512 : (ch + 1) * 512],
                                    start=False,
                                    stop=True,
                                )
                            osb = osbp.tile([128, 512], BF16)
                            nc.scalar.copy(osb[:], op2[:])
                            nc.sync.dma_start(
                                out.ap()[
                                    gt * 128 : (gt + 1) * 128,
                                    ch * 512 : (ch + 1) * 512,
                                ],
                                osb[:],
                            )

    nc.compile()
    _CACHE[key] = nc
    return nc


LAST_RESULT = None


def kernel(x, Wq, bq, Wk, bk, Wv, bv, Wc, bc):
    global LAST_RESULT
    _ensure_hook_shim()
    import ml_dtypes
    from concourse.bass_utils import run_bass_kernel_spmd

    BF = ml_dtypes.bfloat16

    bq = np.asarray(bq, np.float32)
    bk = np.asarray(bk, np.float32)
    bv = np.asarray(bv, np.float32)
    bc = np.asarray(bc, np.float32)
    with_bias = bool(bq.any() or bk.any() or bv.any() or bc.any())
    nc = _build(with_bias)

    x = np.ascontiguousarray(np.asarray(x, dtype=np.float32))
    xt_full = np.ascontiguousarray(x.reshape(ROWS, C).T.astype(BF))  # [C, ROWS]
    wkvt = np.ascontiguousarray(
        np.concatenate(
            [np.asarray(Wk, np.float32).T, np.asarray(Wv, np.float32).T], axis=1
        ).astype(BF)
    )
    wqt = np.ascontiguousarray(np.asarray(Wq, np.float32).T.astype(BF))
    wct = np.ascontiguousarray(np.asarray(Wc, np.float32).T.astype(BF))

    in_maps = []
    for c in range(N_CORES):
        m = {
            "xt": np.ascontiguousarray(xt_full[:, c * RPC : (c + 1) * RPC]),
            "wkvt": wkvt,
            "wqt": wqt,
            "wct": wct,
        }
        if with_bias:
            m["bkv"] = np.concatenate([bk, bv]).reshape(1, 2 * C).astype(BF)
            m["bq"] = bq.reshape(1, C).astype(BF)
            m["bc"] = bc.reshape(1, C).astype(BF)
        in_maps.append(m)

    res = run_bass_kernel_spmd(nc, in_maps, core_ids=list(range(N_CORES)))
    LAST_RESULT = res
    out = np.concatenate(
        [
            np.asarray(res.results[c]["out"]).astype(np.float32)
            for c in range(N_CORES)
        ],
        axis=0,
    )
    return out.reshape(B, T, C)


# revision 16
# speedup vs baseline: 1.0612x; 1.0376x over previous
"""BioLinearAttention (ELU+1 linear attention) on 8 TRN2 NeuronCores.

Sharding: token-parallel. The (B, T) = (4, 4096) grid flattens to 16384 rows;
each core owns 2048 contiguous rows (core c holds batch c//2's half). Each core
computes k/v projections for its rows, accumulates partial kv = k'^T v and
k_sum directly in PSUM across all 16 token tiles, then a pairwise AllReduce
(cores 2b, 2b+1 share batch b) completes the per-batch kv / k_sum. The q
projection for all rows runs while the collective is in flight. Stage C
computes den = q'.k_sum via block-diagonal 2-head matmuls, pre-scales
q~ = q' / den on the DVE, then y^T = kv^T_blockdiag @ q~ (K=128 2-head packed)
and the output projection.

All matmul operands are bf16 (inputs cast host-side); PSUM accumulation stays
fp32. The output is written bf16 and cast back to fp32 on host. DVE element
wise ops run on bf16 SBUF tiles to hit the 2x/4x DVE fast paths; reciprocal
stays fp32 (reciprocal_approx_fast requires it).
"""

import sys
import types

import numpy as np

B, T, C = 4, 4096, 1024
H, D = 16, 64
N_CORES = 8
ROWS = B * T
RPC = ROWS // N_CORES  # rows per core: 2048
NT = RPC // 128  # 128-token tiles per core: 16
NST = RPC // 512  # 512-token super-tiles per core: 4

_CACHE = {}


def _ensure_hook_shim():
    """bass_utils imports antenv.axon_hooks when BASS_TRACE is set; the image
    lacks that module. Provide a no-op shim unless one is already installed."""
    if "antenv.axon_hooks" in sys.modules:
        return
    try:
        import antenv
    except ImportError:
        return
    mod = types.ModuleType("antenv.axon_hooks")
    mod._hook = None
    mod.set_axon_ntff_profile_hook = lambda h: setattr(mod, "_hook", h)
    mod.get_axon_ntff_profile_hook = lambda: mod._hook
    sys.modules["antenv.axon_hooks"] = mod
    antenv.axon_hooks = mod


def _build(with_bias):
    key = ("nc", with_bias)
    if key in _CACHE:
        return _CACHE[key]

    import concourse.bacc as bacc
    import concourse.mybir as mybir
    from concourse.tile import TileContext

    F32 = mybir.dt.float32
    BF16 = mybir.dt.bfloat16
    AF = mybir.ActivationFunctionType

    nc = bacc.Bacc("TRN2", num_devices=N_CORES, debug=False)

    xt = nc.dram_tensor("xt", [C, RPC], BF16, kind="ExternalInput")
    wkvt = nc.dram_tensor("wkvt", [C, 2 * C], BF16, kind="ExternalInput")
    wqt = nc.dram_tensor("wqt", [C, C], BF16, kind="ExternalInput")
    wct = nc.dram_tensor("wct", [C, C], BF16, kind="ExternalInput")
    if with_bias:
        bkv = nc.dram_tensor("bkv", [1, 2 * C], BF16, kind="ExternalInput")
        bq = nc.dram_tensor("bq", [1, C], BF16, kind="ExternalInput")
        bc = nc.dram_tensor("bc", [1, C], BF16, kind="ExternalInput")
    out = nc.dram_tensor("out", [RPC, C], BF16, kind="ExternalOutput")
    # kv (rows 0:64) | k_sum (row 64) partials, layout [d, (h, e)]
    cc_in = nc.dram_tensor("cc_in", [D + 1, C], F32, kind="Internal")
    cc_out = nc.dram_tensor("cc_out", [D + 1, C], F32, kind="Internal")
    groups = [[0, 1], [2, 3], [4, 5], [6, 7]]

    with TileContext(nc) as tc:
        with (
            tc.tile_pool(name="const", bufs=1) as cst,
            tc.tile_pool(name="wts", bufs=1) as wtp,
            tc.tile_pool(name="xres", bufs=1) as xrp,
            tc.tile_pool(name="qres", bufs=1) as qrp,
            tc.tile_pool(name="kvres", bufs=1) as kvp,
        ):
            # ---- persistent SBUF ----
            x_sb = xrp.tile([128, 8, RPC], BF16)  # x^T, c-chunk major
            wkv_sb = wtp.tile([128, 8, 2 * C], BF16)
            wq_sb = wtp.tile([128, 8, C], BF16)
            wc_sb = wtp.tile([128, 8, C], BF16)
            qn_sb = qrp.tile([128, 8, RPC], BF16)  # q' (ELU+1), later q~ in place
            kv2 = kvp.tile([128, 8, 128], BF16)  # block-diag per head pair
            krep2 = kvp.tile([128, 8, 128], BF16)  # block-diag ksum-replicated
            kvt_sb = kvp.tile([D + 1, 8, 128], F32)  # collective result
            ksum_bf = kvp.tile([1, 8, 128], BF16)
            onecol = cst.tile([128, 1], BF16)
            ones_row = cst.tile([1, 512], BF16)
            if with_bias:
                bkv_sb = cst.tile([1, 2 * C], BF16)
                bq_sb = cst.tile([1, C], BF16)
                bc_sb = cst.tile([1, C], BF16)
                nc.sync.dma_start(bkv_sb[:], bkv.ap())
                nc.sync.dma_start(bq_sb[:], bq.ap())
                nc.sync.dma_start(bc_sb[:], bc.ap())

            x_re = xt.ap().rearrange("(c p) t -> p c t", p=128)
            wkv_re = wkvt.ap().rearrange("(c p) n -> p c n", p=128)

            # per-ic granularity for tile 0's deps so the first k-proj matmul
            # can start as soon as its own slice lands (ic-ordered)
            for ic in range(8):
                nc.sync.dma_start(x_sb[:, ic, 0:128], x_re[:, ic, 0:128])
                nc.sync.dma_start(
                    wkv_sb[:, ic, 0:512], wkv_re[:, ic, 0:512]
                )
            for ic in range(8):
                nc.sync.dma_start(
                    wkv_sb[:, ic, 512:1024], wkv_re[:, ic, 512:1024]
                )
            nc.sync.dma_start(x_sb[:, :, 128:256], x_re[:, :, 128:256])
            for nh in range(2, 4):
                nc.sync.dma_start(
                    wkv_sb[:, :, nh * 512 : (nh + 1) * 512],
                    wkv_re[:, :, nh * 512 : (nh + 1) * 512],
                )
            nc.sync.dma_start(x_sb[:, :, 256:512], x_re[:, :, 256:512])
            nc.vector.memset(onecol[:], 1.0)
            nc.vector.memset(ones_row[:], 1.0)
            nc.vector.memset(kv2[:], 0.0)
            nc.vector.memset(krep2[:], 0.0)
            for half in range(2):  # rest of x
                nc.sync.dma_start(
                    x_sb[:, :, 512 + half * 768 : 512 + (half + 1) * 768],
                    x_re[:, :, 512 + half * 768 : 512 + (half + 1) * 768],
                )
            nc.sync.dma_start(
                wq_sb[:], wqt.ap().rearrange("(c p) n -> p c n", p=128)
            )
            nc.sync.dma_start(
                wc_sb[:], wct.ap().rearrange("(c p) n -> p c n", p=128)
            )

            # ---------------- stage A: k/v proj, kv & k_sum PSUM accumulation
            with (
                tc.tile_pool(name="kv1", bufs=2) as kvp1,
                tc.tile_pool(name="el1", bufs=2) as el1,
                tc.tile_pool(name="ps1", bufs=6, space="PSUM") as ps1,
                tc.tile_pool(name="kvacc", bufs=1, space="PSUM") as kvap,
            ):
                kvacc = kvap.tile([D + 1, C], F32)

                def emit_kv_outer(kq0, vq0, t0):
                    st_acc = t0 == 0
                    sp_acc = t0 == NT - 1
                    for h in range(H):
                        # start=True marks the whole 2KB PSUM bank row
                        # pending-zero, so only the first head touching
                        # each bank may set it; later heads' first-tile
                        # writes land on pending bytes and init cleanly.
                        nc.tensor.matmul(
                            kvacc[0:D, h * D : (h + 1) * D],
                            lhsT=kq0[:, h * D : (h + 1) * D],
                            rhs=vq0[:, h * D : (h + 1) * D],
                            start=st_acc and h % 8 == 0,
                            stop=sp_acc,
                            skip_group_check=True,
                        )
                    for nh in range(2):
                        nc.tensor.matmul(
                            kvacc[D : D + 1, nh * 512 : (nh + 1) * 512],
                            lhsT=onecol[:],
                            rhs=kq0[:, nh * 512 : (nh + 1) * 512],
                            start=st_acc,
                            stop=sp_acc,
                            skip_group_check=True,
                        )

                def emit_proj_chunk(tt, half, nh, kq, vq):
                    xtile = x_sb[:, :, tt * 128 : (tt + 1) * 128]
                    sl = slice(half * C + nh * 512, half * C + (nh + 1) * 512)
                    pk = ps1.tile([128, 512], F32, tag="pk")
                    for ic in range(8):
                        nc.tensor.matmul(
                            pk[:],
                            lhsT=xtile[:, ic, :],
                            rhs=wkv_sb[:, ic, sl],
                            start=(ic == 0),
                            stop=(ic == 7 and not with_bias),
                        )
                    if with_bias:
                        nc.tensor.matmul(
                            pk[:],
                            lhsT=ones_row[0:1, 0:128],
                            rhs=bkv_sb[0:1, sl],
                            start=False,
                            stop=True,
                        )
                    osl = slice(nh * 512, (nh + 1) * 512)
                    if half == 0:
                        # elu(x)+1 = relu(x) + exp(min(x, 0))
                        krelu = el1.tile([128, 512], BF16, tag="krelu")
                        nc.scalar.activation(krelu[:], pk[:], AF.Relu)
                        kmin = el1.tile([128, 512], BF16, tag="kmin")
                        nc.vector.tensor_scalar_min(kmin[:], pk[:], 0.0)
                        nc.scalar.activation(kmin[:], kmin[:], AF.Exp)
                        nc.vector.tensor_add(kq[:, osl], krelu[:], kmin[:])
                    else:
                        nc.scalar.copy(vq[:, osl], pk[:])

                # tiles 0/1 k-first: their v-chunks would otherwise block the
                # in-order PE queue on the v-weight DMA while tile 1's k-work
                # (whose weights are already resident) could run
                kqv01 = [
                    (kvp1.tile([128, C], BF16, name=f"kq{t}", tag=f"kq{t}", bufs=1),
                     kvp1.tile([128, C], BF16, name=f"vq{t}", tag=f"vq{t}", bufs=1))
                    for t in range(2)
                ]
                for tt, half in ((0, 0), (1, 0), (0, 1), (1, 1)):
                    for nh in range(2):
                        emit_proj_chunk(tt, half, nh, *kqv01[tt])
                pending = [(kqv01[0][0], kqv01[0][1], 0), (kqv01[1][0], kqv01[1][1], 1)]
                for tt in range(2, NT):
                    emit_kv_outer(*pending.pop(0))
                    kq = kvp1.tile([128, C], BF16, name="kq", tag="kq")
                    vq = kvp1.tile([128, C], BF16, name="vq", tag="vq")
                    for half in range(2):
                        for nh in range(2):
                            emit_proj_chunk(tt, half, nh, kq, vq)
                    pending.append((kq, vq, tt))
                for p in pending:
                    emit_kv_outer(*p)

                # kickoff AllReduce of [kv | k_sum]
                kvs = el1.tile([D + 1, C], F32, tag="kvs", bufs=1)
                nc.scalar.copy(kvs[:], kvacc[:])
                nc.gpsimd.dma_start(cc_in.ap(), kvs[:])
                nc.gpsimd.collective_compute(
                    "AllReduce",
                    mybir.AluOpType.add,
                    replica_groups=groups,
                    ins=[cc_in.ap().opt()],
                    outs=[cc_out.ap().opt()],
                )
                nc.gpsimd.dma_start(
                    kvt_sb[:], cc_out.ap().rearrange("p (c n) -> p c n", c=8)
                )

            # ---------------- stage B: q projection (overlaps the collective)
            with (
                tc.tile_pool(name="el2", bufs=3) as el2,
                tc.tile_pool(name="zpool", bufs=3) as zp,
                tc.tile_pool(name="ytz", bufs=2) as ytzp,
                tc.tile_pool(name="osb", bufs=3) as osbp,
                tc.tile_pool(name="psq", bufs=2, space="PSUM") as psq,
                tc.tile_pool(name="psden", bufs=2, space="PSUM") as psden,
                tc.tile_pool(name="psy", bufs=2, space="PSUM") as psy,
                tc.tile_pool(name="pso", bufs=2, space="PSUM") as pso,
            ):
                for st in range(NST):
                    tsl = slice(st * 512, (st + 1) * 512)
                    for oc in range(8):
                        qp = psq.tile([128, 512], F32)
                        for ic in range(8):
                            nc.tensor.matmul(
                                qp[:],
                                lhsT=wq_sb[:, ic, oc * 128 : (oc + 1) * 128],
                                rhs=x_sb[:, ic, tsl],
                                start=(ic == 0),
                                stop=(ic == 7 and not with_bias),
                            )
                        if with_bias:
                            nc.tensor.matmul(
                                qp[:],
                                lhsT=bq_sb[0:1, oc * 128 : (oc + 1) * 128],
                                rhs=ones_row[0:1, :],
                                start=False,
                                stop=True,
                            )
                        # elu(x)+1 = relu(x) + exp(min(x, 0))
                        qrelu = el2.tile([128, 512], BF16, tag="qrelu")
                        nc.scalar.activation(qrelu[:], qp[:], AF.Relu)
                        qmin = el2.tile([128, 512], BF16, tag="qmin")
                        nc.vector.tensor_scalar_min(qmin[:], qp[:], 0.0)
                        nc.scalar.activation(qmin[:], qmin[:], AF.Exp)
                        nc.vector.tensor_add(
                            qn_sb[:, oc, tsl], qrelu[:], qmin[:]
                        )

                # ---- kv2 / krep2 assembly (waits on the collective) ----
                nc.scalar.copy(ksum_bf[:], kvt_sb[D : D + 1, :, :])
                krp = psden.tile([128, 8, 64], F32, tag="dps")
                for h in range(H):
                    po = (h % 2) * 64
                    # one bank: only the first write per partition half may
                    # set start (bank-granular pending-zero, as above)
                    nc.tensor.matmul(
                        krp[po : po + 64, h // 2, :],
                        lhsT=ksum_bf[0:1, h // 2, po : po + 64],
                        rhs=ones_row[0:1, 0:64],
                        start=h < 2,
                        stop=True,
                        skip_group_check=True,
                    )
                # scatter kv and krep into zero-padded block-diagonal pair form
                for po in range(2):  # even heads -> rows 0:64, odd -> 64:128
                    sl64 = slice(po * 64, po * 64 + 64)
                    nc.vector.tensor_copy(
                        kv2[sl64, :, sl64],
                        kvt_sb[0:D, :, po * 64 : po * 64 + 64],
                    )
                    nc.vector.tensor_copy(
                        krep2[sl64, :, sl64],
                        krp[sl64, :, :],
                    )

                # ---- stage C: den -> q~ = q'/den -> y^T -> c_proj, software
                # pipelined so each supertile's den/recip/mul overlaps the
                # previous supertile's c_proj instead of bunching on the DVE
                def emit_den(st):
                    tsl = slice(st * 512, (st + 1) * 512)
                    for j in range(8):
                        dps = psden.tile([128, 512], F32)
                        nc.tensor.matmul(
                            dps[:],
                            lhsT=krep2[:, j, :],
                            rhs=qn_sb[:, j, tsl],
                            start=True,
                            stop=True,
                        )
                        z32 = zp.tile([128, 512], F32, tag="z32")
                        nc.vector.reciprocal_approx_fast(z32[:], dps[:])
                        z16 = zp.tile([128, 512], BF16, tag="z16")
                        nc.scalar.copy(z16[:], z32[:])
                        nc.vector.tensor_mul(
                            qn_sb[:, j, tsl], qn_sb[:, j, tsl], z16[:]
                        )

                emit_den(0)
                for st in range(NST):
                    tsl = slice(st * 512, (st + 1) * 512)
                    ytz = ytzp.tile([128, 8, 512], BF16)
                    for j in range(8):
                        yps = psy.tile([128, 512], F32)
                        nc.tensor.matmul(
                            yps[:],
                            lhsT=kv2[:, j, :],
                            rhs=qn_sb[:, j, tsl],
                            start=True,
                            stop=True,
                        )
                        nc.scalar.copy(ytz[:, j, :], yps[:])
                    if st + 1 < NST:
                        emit_den(st + 1)
                    for k in range(4):
                        gt = st * 4 + k
                        for ch in range(2):
                            op2 = pso.tile([128, 512], F32)
                            for oc2 in range(8):
                                nc.tensor.matmul(
                                    op2[:],
                                    lhsT=ytz[:, oc2, k * 128 : (k + 1) * 128],
                                    rhs=wc_sb[:, oc2, ch * 512 : (ch + 1) * 512],
                                    start=(oc2 == 0),
                                    stop=(oc2 == 7 and not with_bias),
                                )
                            if with_bias:
                                nc.tensor.matmul(
                                    op2[:],
                                    lhsT=ones_row[0:1, 0:128],
                                    rhs=bc_sb[0:1, ch * 512 : (ch + 1) * 512],
                                    start=False,
                                    stop=True,
                                )
                            osb = osbp.tile([128, 512], BF16)
                            nc.scalar.copy(osb[:], op2[:])
                            nc.sync.dma_start(
                                out.ap()[
                                    gt * 128 : (gt + 1) * 128,
                                    ch * 512 : (ch + 1) * 512,
                                ],
                                osb[:],
                            )

    nc.compile()
    _CACHE[key] = nc
    return nc


LAST_RESULT = None


def kernel(x, Wq, bq, Wk, bk, Wv, bv, Wc, bc):
    global LAST_RESULT
    _ensure_hook_shim()
    import ml_dtypes
    from concourse.bass_utils import run_bass_kernel_spmd

    BF = ml_dtypes.bfloat16

    bq = np.asarray(bq, np.float32)
    bk = np.asarray(bk, np.float32)
    bv = np.asarray(bv, np.float32)
    bc = np.asarray(bc, np.float32)
    with_bias = bool(bq.any() or bk.any() or bv.any() or bc.any())
    nc = _build(with_bias)

    x = np.ascontiguousarray(np.asarray(x, dtype=np.float32))
    xt_full = np.ascontiguousarray(x.reshape(ROWS, C).T.astype(BF))  # [C, ROWS]
    wkvt = np.ascontiguousarray(
        np.concatenate(
            [np.asarray(Wk, np.float32).T, np.asarray(Wv, np.float32).T], axis=1
        ).astype(BF)
    )
    wqt = np.ascontiguousarray(np.asarray(Wq, np.float32).T.astype(BF))
    wct = np.ascontiguousarray(np.asarray(Wc, np.float32).T.astype(BF))

    in_maps = []
    for c in range(N_CORES):
        m = {
            "xt": np.ascontiguousarray(xt_full[:, c * RPC : (c + 1) * RPC]),
            "wkvt": wkvt,
            "wqt": wqt,
            "wct": wct,
        }
        if with_bias:
            m["bkv"] = np.concatenate([bk, bv]).reshape(1, 2 * C).astype(BF)
            m["bq"] = bq.reshape(1, C).astype(BF)
            m["bc"] = bc.reshape(1, C).astype(BF)
        in_maps.append(m)

    res = run_bass_kernel_spmd(nc, in_maps, core_ids=list(range(N_CORES)))
    LAST_RESULT = res
    out = np.concatenate(
        [
            np.asarray(res.results[c]["out"]).astype(np.float32)
            for c in range(N_CORES)
        ],
        axis=0,
    )
    return out.reshape(B, T, C)
